# revision 12
# baseline (speedup 1.0000x reference)
"""Trainium2 Bass kernel for a dense transformer block (cross-attention + FFN).

Problem: nn_MAB (B=4, nq=nk=1024, D=1024, H=16, HD=64), fp32 in/out.

Sharding: fully data-parallel, zero collectives. 8 cores = 4 batches x 2
query-halves; each core computes 512 query rows of one batch end-to-end
(K/V projections for a batch are duplicated across its 2 cores).

v2 design (vs fp32r baseline): all matmul inputs bf16 (halves HBM traffic,
which dominates on HW), contiguous host-prepped DRAM layouts so every load
is one big DMA, loads spread across both HWDGE queues (SP + Act), softmax
denominator merged into the attnV matmul via ones-columns packed into the
lhsT ([V|1] even head, [1|V] odd head), key mask folded into the V tile
rows (multiplicative - zeroes numerator and denominator contributions),
exp applied in-place on SBUF in one N=4096 ACT op per head, and the pair
loop software-pipelined (attnV lags scores by one pair) so the PE never
waits on the Activation engine.

Layout is transposed [feature, token] throughout; the host un-transposes
on gather.
"""

import numpy as np
import ml_dtypes

import concourse.bass as bass
import concourse.mybir as mybir
import concourse.tile as tile
from concourse import bacc
from concourse.bass_utils import run_bass_kernel_spmd

F32 = mybir.dt.float32
F32R = mybir.dt.float32r
BF16 = mybir.dt.bfloat16
AF = mybir.ActivationFunctionType

D = 1024          # model dim
P = 128           # partitions
NJ = D // P       # feature tiles (8)
NQ = 512          # queries per core
NT = 1024         # keys per core
H = 16
HD = 64
NPAIR = H // 2    # head pairs (8)
NKT = NT // P     # key tiles (8)
EPS = 1e-5
BF = ml_dtypes.bfloat16


def build_nc() -> bass.Bass:
    nc = bacc.Bacc("TRN2", target_bir_lowering=False, debug=False)

    # ---- DRAM I/O (per-core shards; host prepares layouts) ----
    xt = nc.dram_tensor("xt", [P, NJ * NQ], BF16, kind="ExternalInput")[:]
    yt0 = nc.dram_tensor("yt0", [P, NJ * 512], BF16, kind="ExternalInput")[:]
    yt1 = nc.dram_tensor("yt1", [P, NJ * 512], BF16, kind="ExternalInput")[:]
    wk = nc.dram_tensor("wk", [P, NPAIR * NJ * P], BF16, kind="ExternalInput")[:]
    wq = nc.dram_tensor("wq", [P, NPAIR * NJ * P], BF16, kind="ExternalInput")[:]
    wv = nc.dram_tensor("wv", [P, NJ * D], BF16, kind="ExternalInput")[:]
    wo = nc.dram_tensor("wo", [P, NJ * NJ * P], BF16, kind="ExternalInput")[:]
    w1 = nc.dram_tensor("w1", [P, NJ * NJ * P], BF16, kind="ExternalInput")[:]
    w2 = nc.dram_tensor("w2", [P, NJ * NJ * P], BF16, kind="ExternalInput")[:]
    # vecs cols: 0:8 mask01, 8:16 ln1_g, 16:24 ln1_b, 24:32 ln2_g,
    # 32:40 ln2_b, 40:48 b1, 48:56 b2
    vecs = nc.dram_tensor("vecs", [P, 56], F32, kind="ExternalInput")[:]
    outt = nc.dram_tensor("outt", [P, NJ * NQ], F32, kind="ExternalOutput")[:]

    with tile.TileContext(nc) as tc, \
         nc.allow_low_precision(reason="bf16 matmul path, tol 2e-2"):
        with tc.tile_pool(name="persist", bufs=1) as persist, \
             tc.tile_pool(name="psum", bufs=2, space="PSUM") as pp:

            def psum1(name):
                return pp.tile([P, NQ], F32, tag="ps1", name=name)

            with tc.tile_pool(name="attn", bufs=1) as attn:
                # ---- load everything (big contiguous DMAs, both queues) ----
                yt_sb = attn.tile([P, 2, NJ, 512], BF16)  # [p, key-half, k, t]
                nc.sync.dma_start(yt_sb[:, 0], yt0)
                wv_sb = attn.tile([P, NJ, D], BF16)       # [p, k, f]
                nc.scalar.dma_start(wv_sb, wv)
                wk_sb = attn.tile([P, NPAIR, NJ, P], BF16)
                nc.sync.dma_start(wk_sb, wk)
                vecs_sb = persist.tile([P, 56], F32)
                nc.scalar.dma_start(vecs_sb, vecs)
                xt_sb = persist.tile([P, NJ, NQ], BF16)
                nc.scalar.dma_start(xt_sb, xt)
                nc.sync.dma_start(yt_sb[:, 1], yt1)
                wq_sb = attn.tile([P, NPAIR, NJ, P], BF16)
                nc.scalar.dma_start(wq_sb, wq)
                wo_sb = persist.tile([P, NJ, NJ, P], BF16)
                nc.sync.dma_start(wo_sb, wo)
                w1_sb = persist.tile([P, NJ, NJ, P], BF16)
                nc.scalar.dma_start(w1_sb, w1)
                w2_sb = persist.tile([P, NJ, NJ, P], BF16)
                nc.sync.dma_start(w2_sb, w2)

                mask01 = vecs_sb[:, 0:8]

                # ---- constants ----
                cst = persist.tile([P, P], F32)
                lnw = persist.tile([P, P], F32R)      # 1/D for LN mean matmul
                nc.vector.memset(cst, 1.0 / D)
                nc.vector.tensor_copy(lnw, cst)
                lnwb = persist.tile([P, P], BF16)     # 1/D for LN var matmul
                nc.vector.memset(lnwb, 1.0 / D)
                eps_sb = persist.tile([P, 1], F32)
                nc.vector.memset(eps_sb, EPS)

                # attention head outputs, feature-major, natural head order
                outT = persist.tile([P, NJ, NQ], BF16)

                # va: [p(key%128), kt, pair, parity, colhalf, 64]
                # parity 0 head: [V|1];  parity 1 head: [1|V]
                va = attn.tile([P, NKT, NPAIR, 2, 2, HD], BF16)
                nc.vector.memset(va[:, :, :, 0, 1, :], 1.0)
                nc.vector.memset(va[:, :, :, 1, 0, :], 1.0)

                # ---- V = Y @ Wv.T, keys on partitions ----
                with tc.tile_pool(name="vps", bufs=2, space="PSUM") as vpp:
                    for tt in range(NKT):
                        h, t0 = divmod(tt * P, 512)
                        for ci in range(2):
                            ps = vpp.tile([P, 4, 2, HD], F32, tag="psv",
                                          name="ps_v")
                            for k in range(NJ):
                                nc.tensor.matmul(
                                    ps, yt_sb[:, h, k, t0:t0 + P],
                                    wv_sb[:, k, ci * 512:(ci + 1) * 512],
                                    start=(k == 0), stop=(k == NJ - 1),
                                )
                            jp = slice(4 * ci, 4 * ci + 4)
                            # eviction with the key mask folded in for free
                            nc.vector.tensor_scalar(
                                va[:, tt, jp, 0, 0, :], ps[:, :, 0, :],
                                mask01[:, tt:tt + 1], None,
                                mybir.AluOpType.mult)
                            nc.vector.tensor_scalar(
                                va[:, tt, jp, 1, 1, :], ps[:, :, 1, :],
                                mask01[:, tt:tt + 1], None,
                                mybir.AluOpType.mult)
                        # mask the ones columns (denominator contributions)
                        nc.vector.tensor_scalar(
                            va[:, tt, :, 0, 1, :], va[:, tt, :, 0, 1, :],
                            mask01[:, tt:tt + 1], None, mybir.AluOpType.mult)
                        nc.vector.tensor_scalar(
                            va[:, tt, :, 1, 0, :], va[:, tt, :, 1, 0, :],
                            mask01[:, tt:tt + 1], None, mybir.AluOpType.mult)

                # ---- pair loop, software-pipelined: stage A (KQ, scores,
                # exp) for pair j runs while stage B (attnV, divide) handles
                # pair j-1 ----
                with tc.tile_pool(name="qk", bufs=2) as qkp, \
                     tc.tile_pool(name="exp", bufs=2) as ep, \
                     tc.tile_pool(name="dn", bufs=2) as dnp, \
                     tc.tile_pool(name="sps", bufs=2, space="PSUM") as spp, \
                     tc.tile_pool(name="aps", bufs=2, space="PSUM") as app:

                    def stage_a(j):
                        kt_j = qkp.tile([P, NT], BF16, tag="kt", name="kt_j")
                        for h in range(2):
                            ps = psum1("ps_k")
                            for k in range(NJ):
                                nc.tensor.matmul(
                                    ps, wk_sb[:, j, k, :], yt_sb[:, h, k, :],
                                    start=(k == 0), stop=(k == NJ - 1),
                                )
                            nc.vector.tensor_copy(
                                kt_j[:, h * 512:(h + 1) * 512], ps)
                        qt_j = qkp.tile([P, NQ], BF16, tag="qt", name="qt_j")
                        ps = psum1("ps_q")
                        for k in range(NJ):
                            nc.tensor.matmul(
                                ps, wq_sb[:, j, k, :], xt_sb[:, k, :],
                                start=(k == 0), stop=(k == NJ - 1),
                            )
                        nc.vector.tensor_copy(qt_j, ps)

                        # scoresT, evicted to SBUF; exp in-place (one big
                        # ACT op per head)
                        exp_e = ep.tile([P, NKT, NQ], BF16, tag="ex",
                                        name="exp_e")
                        exp_o = ep.tile([P, NKT, NQ], BF16, tag="ex",
                                        name="exp_o")
                        for kp in range(NKT // 2):
                            ps_se = spp.tile([P, 2 * NQ], F32, tag="ss",
                                             name="ps_se")
                            ps_so = spp.tile([P, 2 * NQ], F32, tag="ss",
                                             name="ps_so")
                            for u in range(2):
                                ks = slice((2 * kp + u) * P,
                                           (2 * kp + u + 1) * P)
                                us = slice(u * NQ, (u + 1) * NQ)
                                nc.tensor.matmul(
                                    ps_se[:, us], kt_j[0:HD, ks],
                                    qt_j[0:HD, :], start=True, stop=True)
                                nc.tensor.matmul(
                                    ps_so[:, us], kt_j[HD:P, ks],
                                    qt_j[HD:P, :], start=True, stop=True)
                            nc.scalar.activation(
                                exp_e[:, 2 * kp:2 * kp + 2, :], ps_se, AF.Exp)
                            nc.scalar.activation(
                                exp_o[:, 2 * kp:2 * kp + 2, :], ps_so, AF.Exp)
                        return exp_e, exp_o

                    def stage_b(j, exp_e, exp_o):
                        ps_e = app.tile([P, NQ], F32, tag="av", name="ps_ae")
                        ps_o = app.tile([P, NQ], F32, tag="av", name="ps_ao")
                        for kt in range(NKT):
                            st, sp = kt == 0, kt == NKT - 1
                            nc.tensor.matmul(
                                ps_e, va[:, kt, j, 0], exp_e[:, kt, :],
                                start=st, stop=sp)
                            nc.tensor.matmul(
                                ps_o, va[:, kt, j, 1], exp_o[:, kt, :],
                                start=st, stop=sp)
                        # even head: data 0:64, denom 64:128
                        # odd head:  denom 0:64, data 64:128
                        rc = dnp.tile([P, NQ], BF16, tag="rc", name="rc")
                        nc.vector.reciprocal(rc[0:HD, :], ps_o[0:HD, :])
                        nc.vector.reciprocal(rc[HD:P, :], ps_e[HD:P, :])
                        rc2 = dnp.tile([P, NQ], BF16, tag="rc2", name="rc2")
                        nc.sync.dma_start(rc2[0:HD, :], rc[HD:P, :])
                        nc.scalar.dma_start(rc2[HD:P, :], rc[0:HD, :])
                        nc.vector.tensor_mul(
                            outT[0:HD, j, :], ps_e[0:HD, :], rc2[0:HD, :])
                        nc.vector.tensor_mul(
                            outT[HD:P, j, :], ps_o[HD:P, :], rc2[HD:P, :])

                    prev = None
                    for j in range(NPAIR):
                        cur = stage_a(j)
                        if prev is not None:
                            stage_b(j - 1, *prev)
                        prev = cur
                    stage_b(NPAIR - 1, *prev)

            # ---- O-projection + residual, then LN1 / FFN / LN2 ----
            with tc.tile_pool(name="tail", bufs=1) as tl, \
                 tc.tile_pool(name="ln_pool", bufs=6) as lnp:

                def ln_transposed(x_sb, xsq_sb, gv, bv, dest):
                    """LayerNorm over the partition(feature) axis of
                    x_sb [P, NJ, NQ]; writes dest[:, j, :]."""
                    for jj in range(NJ):
                        nc.scalar.activation(
                            xsq_sb[:, jj, :], x_sb[:, jj, :], AF.Square)
                    ps_m = psum1("ps_m")
                    ps_v = psum1("ps_v2")
                    for jj in range(NJ):
                        nc.tensor.matmul(
                            ps_m, lnw, x_sb[:, jj, :],
                            start=(jj == 0), stop=(jj == NJ - 1))
                    for jj in range(NJ):
                        nc.tensor.matmul(
                            ps_v, lnwb, xsq_sb[:, jj, :],
                            start=(jj == 0), stop=(jj == NJ - 1))
                    mean = lnp.tile([P, NQ], F32, tag="lnt", name="mean")
                    nc.vector.tensor_copy(mean, ps_m)
                    msq = lnp.tile([P, NQ], F32, tag="lnt", name="msq")
                    nc.vector.tensor_mul(msq, mean, mean)
                    var = lnp.tile([P, NQ], F32, tag="lnt", name="var")
                    nc.vector.tensor_tensor(
                        var, ps_v, msq, mybir.AluOpType.subtract)
                    sd = lnp.tile([P, NQ], F32, tag="lnt", name="sd")
                    nc.scalar.activation(sd, var, AF.Sqrt, bias=eps_sb,
                                         scale=1.0)
                    rstd = lnp.tile([P, NQ], F32, tag="lnt", name="rstd")
                    nc.vector.reciprocal(rstd, sd)
                    mrs = lnp.tile([P, NQ], F32, tag="lnt", name="mrs")
                    nc.vector.tensor_mul(mrs, mean, rstd)
                    for jj in range(NJ):
                        t = lnp.tile([P, NQ], F32, tag="lnt", name="t")
                        nc.vector.tensor_mul(t, x_sb[:, jj, :], rstd)
                        nc.vector.tensor_tensor(
                            t, t, mrs, mybir.AluOpType.subtract)
                        nc.vector.tensor_scalar(
                            dest[:, jj, :], t,
                            gv[:, jj:jj + 1], bv[:, jj:jj + 1],
                            mybir.AluOpType.mult, mybir.AluOpType.add)

                x1 = tl.tile([P, NJ, NQ], F32R)     # X + attn_out (transposed)
                for m in range(NJ):
                    ps = psum1("ps_z")
                    for g in range(NJ):
                        nc.tensor.matmul(
                            ps, wo_sb[:, m, g, :], outT[:, g, :],
                            start=(g == 0), stop=(g == NJ - 1))
                    nc.vector.tensor_add(x1[:, m, :], ps, xt_sb[:, m, :])

                xsq = tl.tile([P, NJ, NQ], BF16)
                hT = tl.tile([P, NJ, NQ], BF16)
                ln_transposed(x1, xsq, vecs_sb[:, 8:16], vecs_sb[:, 16:24], hT)

                ff1 = tl.tile([P, NJ, NQ], BF16)
                for m in range(NJ):
                    ps = psum1("ps_f1")
                    for k in range(NJ):
                        nc.tensor.matmul(
                            ps, w1_sb[:, m, k, :], hT[:, k, :],
                            start=(k == 0), stop=(k == NJ - 1))
                    nc.scalar.activation(
                        ff1[:, m, :], ps, AF.Relu,
                        bias=vecs_sb[:, 40 + m:41 + m], scale=1.0)

                x2 = tl.tile([P, NJ, NQ], F32R)     # H + FFN (transposed)
                for m in range(NJ):
                    ps = psum1("ps_f2")
                    for k in range(NJ):
                        nc.tensor.matmul(
                            ps, w2_sb[:, m, k, :], ff1[:, k, :],
                            start=(k == 0), stop=(k == NJ - 1))
                    nc.vector.scalar_tensor_tensor(
                        x2[:, m, :], ps, vecs_sb[:, 48 + m:49 + m],
                        hT[:, m, :],
                        op0=mybir.AluOpType.add, op1=mybir.AluOpType.add)

                o_sb = tl.tile([P, NJ, NQ], F32)
                ln_transposed(x2, xsq, vecs_sb[:, 24:32], vecs_sb[:, 32:40],
                              o_sb)
                ot = outt.rearrange("p (j q) -> p j q", q=NQ)
                for jj in range(NJ):
                    eng = nc.sync if jj % 2 == 0 else nc.scalar
                    eng.dma_start(ot[:, jj, :], o_sb[:, jj, :])

    nc.compile()
    return nc


_NC_CACHE: dict = {}


def _get_nc() -> bass.Bass:
    if "nc" not in _NC_CACHE:
        _NC_CACHE["nc"] = build_nc()
    return _NC_CACHE["nc"]


def _prep_inputs(X, Y, mask_y, Wq, Wk, Wv, Wo, ln1_g, ln1_b, ln2_g, ln2_b,
                 W1, b1, W2, b2):
    f32 = lambda a: np.asarray(a, dtype=np.float32)
    bf = lambda a: np.ascontiguousarray(a).astype(BF)
    X, Y = f32(X), f32(Y)
    mask_y = np.asarray(mask_y)

    # weights pair-tiled: w_a[p, j, k, m] = W[j*128+m, k*128+p]
    def pair_tiles(W):
        return bf(f32(W).reshape(NJ, P, NJ, P).transpose(3, 0, 2, 1)
                  .reshape(P, NJ * NJ * P))

    wk_a = pair_tiles(Wk)
    wq_a = pair_tiles(f32(Wq) / np.float32(8.0))
    wo_a = pair_tiles(Wo)
    w1_a = pair_tiles(W1)
    w2_a = pair_tiles(W2)
    # wv_a[p, k, f] = Wv[f, k*128+p]
    wv_a = bf(f32(Wv).reshape(D, NJ, P).transpose(2, 1, 0).reshape(P, NJ * D))

    def cols8(v):  # [1024] -> [128, 8]
        return f32(v).reshape(NJ, P).T

    in_maps = []
    for core in range(8):
        b, half = divmod(core, 2)
        q0 = half * NQ
        vecs = np.empty((P, 56), np.float32)
        vecs[:, 0:8] = cols8(mask_y[b].astype(np.float32))
        vecs[:, 8:16] = cols8(ln1_g)
        vecs[:, 16:24] = cols8(ln1_b)
        vecs[:, 24:32] = cols8(ln2_g)
        vecs[:, 32:40] = cols8(ln2_b)
        vecs[:, 40:48] = cols8(b1)
        vecs[:, 48:56] = cols8(b2)
        # xt[p, j, q] = X[b, q0+q, j*128+p]
        xt_a = bf(X[b, q0:q0 + NQ, :].reshape(NQ, NJ, P).transpose(2, 1, 0)
                  .reshape(P, NJ * NQ))
        # yt_h[p, k, t] = Y[b, h*512+t, k*128+p]
        yb = Y[b].reshape(2, 512, NJ, P).transpose(0, 3, 2, 1)
        m = dict(
            wk=wk_a, wq=wq_a, wv=wv_a, wo=wo_a, w1=w1_a, w2=w2_a,
            vecs=vecs, xt=xt_a,
            yt0=bf(yb[0].reshape(P, NJ * 512)),
            yt1=bf(yb[1].reshape(P, NJ * 512)),
        )
        in_maps.append(m)
    return in_maps


def _run(in_maps, **kwargs):
    return run_bass_kernel_spmd(_get_nc(), in_maps, core_ids=list(range(8)),
                                **kwargs)


def kernel(**inputs) -> np.ndarray:
    in_maps = _prep_inputs(**inputs)
    res = _run(in_maps)
    B, nq = 4, 1024
    out = np.empty((B, nq, D), dtype=np.float32)
    for core in range(8):
        b, half = divmod(core, 2)
        q0 = half * NQ
        # outt[p, j, q] -> out[q, j*128+p]
        o = res.results[core]["outt"].reshape(P, NJ, NQ)
        out[b, q0:q0 + NQ, :] = o.transpose(2, 1, 0).reshape(NQ, D)
    return out
